# revision 28
# baseline (speedup 1.0000x reference)
"""Trainium2 Bass kernel for nn_ConditionalJiTBlock (DiT-style block with
AdaLN modulation, self-attention, cross-attention and SwiGLU FFN).

Sharding: 8 NeuronCores = 4 batch elements x 2 token-halves. Each core
computes its 512 query tokens end-to-end with zero collectives; the K/V
projections (which need all 1024 tokens of the batch element) are
replicated within each pair of cores. SPMD safety: the host permutes each
core's token axis so the core's local tokens are always columns 0..511 of
the on-chip tensors (attention is permutation-invariant over key tokens).

Layout: activations are feature-major on chip (features on partitions,
tokens on the free axis). Projections run as fp8 DoubleRow matmuls;
weights are host-prepacked into device-contiguous [128, cols] megas
(col order kp,two,f) so each load is one or two large contiguous DMAs.
Attention scores for a head pair are emitted as two row-tiled concurrent
K=64 matmuls into one 2-bank [128, 1024] PSUM tile, exponentiated by a
single wide ACT op. Softmax denominators come from an interleaved
ones-column in the token-major V tiles. Startup: a short PE warm-up
burst opens the HAM clock gate while the critical DMAs (cvec, ada
groups 0-1, xt, wqkv) stream in, issued across four engine queues.
"""

import numpy as np
import ml_dtypes

BF16 = ml_dtypes.bfloat16
F8 = ml_dtypes.float8_e4m3

B, N, M, D, H, HD = 4, 1024, 1024, 1024, 16, 64
MH = 2730
MHP = 2816          # MH padded to 22*128
EPS = 1e-6
NCORES = 8
T = 512             # local query tokens per core
DT = D // 128       # 8
KP = DT // 2        # 4 k-tile pairs for DoubleRow
FHT = MHP // 128    # 22
NMOD = 9
ATT_SCALE = HD ** -0.5
WS = 1024.0         # fp8 weight pre-scale (power of 2)
IWS = 1.0 / WS

# cf32 mega-constant column map: name -> (col0, ncols)
_CF32 = {}
_c = 0
for _nm, _nc_ in (("adab", NMOD * DT), ("n1w", DT), ("ncw", DT), ("n2w", DT),
                  ("qkvb", 3 * DT), ("obf", DT), ("cqb", DT), ("ckb", DT),
                  ("cobf", DT), ("b1f", FHT), ("b2f", FHT), ("b3f", DT)):
    _CF32[_nm] = (_c, _nc_)
    _c += _nc_
CF32_COLS = _c

W1GRP = [(0, 4), (4, 4), (8, 4), (12, 4), (16, 4), (20, 2)]   # w1/w2 f-tile groups


# ==========================================================================
# device graph
# ==========================================================================

def build_graph():
    import concourse.bacc as bacc
    import concourse.mybir as mybir
    import concourse.tile as tile

    F32 = mybir.dt.float32
    BT = mybir.dt.bfloat16
    E4 = mybir.dt.float8e4

    nc = bacc.Bacc("TRN2", target_bir_lowering=False, debug=False,
                   num_devices=NCORES)

    def din(name, shape, dtype):
        return nc.dram_tensor(name, shape, dtype, kind="ExternalInput").ap()

    p = {}
    # activations (host-prepacked feature-major / kp-major layouts)
    for k in range(DT):
        p[f"xt{k}"] = din(f"xt{k}", [128, N], BT)       # x[b].T tile k
    p["xres"] = din("xres", [D, T], F32)                # f32 residual columns
    p["cvec"] = din("cvec", [128, DT], F32)             # c[b] feature-major
    p["srct8"] = din("srct8", [128, 2 * KP * M], E4)    # (kp, j, m)
    # weights: fp8 megas, col order (kp, two, f) per group
    p["ada"] = din("ada", [128, 18 * 4096], E4)         # 18 half-groups
    p["wqkv"] = din("wqkv", [128, 3 * 8192], E4)
    p["wo"] = din("wo", [128, 8192], E4)
    p["wcq"] = din("wcq", [128, 8192], E4)
    p["wckv"] = din("wckv", [128, 2 * 8192], E4)
    p["wco"] = din("wco", [128, 8192], E4)
    p["w1"] = din("w1", [128, KP * 2 * MHP], E4)        # groups of 8 f-tiles
    p["w2"] = din("w2", [128, KP * 2 * MHP], E4)
    p["w3"] = din("w3", [128, 2 * 11 * 1024], E4)       # (half, kp, two, f512)
    # constants
    p["cf32"] = din("cf32", [128, CF32_COLS], F32)
    p["cbf"] = din("cbf", [128, 272], BT)               # ones128 | bd16 | eye16
    p["sels"] = din("sels", [16, 3 * D], BT)            # qsel|cqsel|bsel

    p["out"] = nc.dram_tensor("out", [D, T], F32, kind="ExternalOutput").ap()

    with tile.TileContext(nc) as tc:
        _emit(nc, tc, p, mybir)
    nc.compile()
    return nc


def _emit(nc, tc, p, mybir):
    ALU = mybir.AluOpType
    ACTF = mybir.ActivationFunctionType
    PM = mybir.MatmulPerfMode
    F32 = mybir.dt.float32
    BT = mybir.dt.bfloat16
    E4 = mybir.dt.float8e4

    pg = tc.alloc_tile_pool(name="pg", bufs=1)
    ps = tc.alloc_tile_pool(name="ps", bufs=8, space="PSUM")

    # ---- psum allocators: 2x [128,1024] (2 banks) + 4x [128,512] ----
    def psum2(name):
        return ps.tile([128, 1024], F32, tag="ps2", name=name, bufs=2)

    def psum(name):
        return ps.tile([128, 512], F32, tag="ps1", name=name, bufs=4)

    # ---- sbuf allocators ----
    def kb(name):     # bf16 [128,1024] xt/k tiles
        return pg.tile([128, 1024], BT, tag="kb", name=name, bufs=16)

    def vb(name):     # bf16 [128,1040] v tiles (ones cols interleaved)
        return pg.tile([128, 1040], BT, tag="vb", name=name, bufs=16)

    def xf(name):     # f32 [128, T] residual-stream tiles
        return pg.tile([128, T], F32, tag="xf", name=name, bufs=16)

    def qt(name):     # bf16 [128, T] q tiles
        return pg.tile([128, T], BT, tag="qt", name=name, bufs=8)

    def wg4(name):    # fp8 packed weight stream tiles (2 kp each)
        return pg.tile([128, 4096], E4, tag="wg4", name=name, bufs=3)

    def wga(name):    # fp8 ada quarter-group stream tiles (2 kp)
        return pg.tile([128, 2048], E4, tag="wga", name=name, bufs=2)

    def wff(name):    # fp8 w1/w2 stream tiles (half-groups of 2 kp)
        return pg.tile([128, 2048], E4, tag="wff", name=name, bufs=4)

    def w3m(name):    # fp8 w3 chunk tiles
        return pg.tile([128, 4096], E4, tag="w3m", name=name, bufs=3)

    def pairw(name):  # fp8 k-pair tiles, 1024 tokens (xn1)
        return pg.tile([128, 2048], E4, tag="pairw", name=name, bufs=4)

    def pair8(name, wid):  # fp8 k-pair tiles, 512 tokens (xn/o/h)
        return pg.tile([128, 1024], E4, tag="pair8", name=name, bufs=11)

    def ptile(name):  # wide exp(p) tiles
        return pg.tile([128, 1024], BT, tag="pt", name=name, bufs=3)

    def sqt(name):    # square scratch
        return pg.tile([128, 512], BT, tag="sq", name=name, bufs=3)

    def scratch4k(name, rows=128, wid=1024):  # f32 scratch (rr/ssq/den)
        return pg.tile([rows, wid], F32, tag="s4k", name=name, bufs=2)

    def scrbf(name, rows=16, wid=512):
        return pg.tile([rows, wid], BT, tag="sbf", name=name, bufs=2)

    def sq_engine(i):
        return nc.gpsimd if i % 2 == 1 else nc.vector

    # =====================================================================
    # Stage 0: PE warm-up burst + input DMAs in critical-path order,
    # spread across four engine queues; then silu(c).
    # =====================================================================
    warm = pg.tile([128, 512], BT, tag="sq", name="warm", bufs=3)
    nc.vector.memset(warm[:], 0.0)
    dmy = pg.tile([1, 4], F32, tag="dmy", name="dmy")
    wps = psum("warmps")
    for i in range(20):
        nc.tensor.matmul(wps[:], warm[:, 0:128], warm[:], start=True,
                         stop=True)
    nc.vector.tensor_copy(dmy[:, 0:1], wps[0:1, 0:1])

    # critical first: cvec (gates silu(c) -> ada matvec)
    cv = pg.tile([128, DT], F32, tag="cv", name="cv")
    nc.sync.dma_start(cv[:], p["cvec"][:])
    cf32 = pg.tile([128, CF32_COLS], F32, tag="cf32", name="cf32")
    nc.scalar.dma_start(cf32[:], p["cf32"][:])

    cst = {nm: cf32[:, c0:c0 + ncol] for nm, (c0, ncol) in _CF32.items()}

    c_eps = pg.tile([128, 2], F32, tag="c_eps", name="c_eps")
    nc.gpsimd.memset(c_eps[:, 0:1], EPS)
    nc.gpsimd.memset(c_eps[:, 1:2], HD * EPS)
    dmask = pg.tile([1, 256], BT, tag="dmask", name="dmask")
    nc.gpsimd.memset(dmask[:], 0.0)
    nc.gpsimd.memset(dmask[0:1, 0:64], 1.0 / 64.0)
    nc.gpsimd.memset(dmask[0:1, 192:256], 1.0 / 64.0)

    def prewarm(func):
        nc.scalar.activation(dmy[:, 1:2], c_eps[0:1, 0:1], func)

    # weight mega loader: returns per-kp [128, 2, F] views
    def load_w8(dram, col0, ncols, tagname, eng=None, alloc=None):
        F = ncols // (KP * 2)
        alloc = alloc or wg4
        engs = eng or (nc.sync, nc.sync)
        tiles = []
        for h in range(2):
            t = alloc(f"{tagname}_{h}")
            engs[h].dma_start(t[:, 0:ncols // 2],
                              dram[:, col0 + h * ncols // 2:
                                   col0 + (h + 1) * ncols // 2])
            tiles.append(t)
        views = []
        for kp in range(KP):
            base = (kp % 2) * (2 * F)
            views.append(tiles[kp // 2][:, base:base + 2 * F]
                         .rearrange("p (two f) -> p two f", two=2))
        return views

    # silu(c) -> fp8 DoubleRow stationary (emitted before the bulk DMA
    # issues so the scalar queue reaches the sigmoid immediately)
    sc = pg.tile([128, DT], BT, tag="sc", name="sc")
    nc.scalar.activation(sc[:], cv[:], ACTF.Sigmoid)
    prewarm(ACTF.Sqrt)
    nc.vector.tensor_tensor(sc[:], sc[:], cv[:], ALU.mult)
    sc8 = pg.tile([128, 128], E4, tag="sc8", name="sc8")
    sc8v = sc8[:].rearrange("p (kp two s) -> p kp two s", two=2, s=16)
    nc.vector.tensor_copy(sc8v[:, :, :, 0:1],
                          sc[:].rearrange("p (kp two) -> p kp two", two=2)
                          .rearrange("p kp two -> p kp two ()"))

    xt_sb = [kb(f"xt{k}") for k in range(DT)]

    def load_xt_consts():
        for k in range(DT):
            nc.scalar.dma_start(xt_sb[k][:, 0:N], p[f"xt{k}"][:])
        nc.scalar.dma_start(cbf[:], p["cbf"][:])
        nc.scalar.dma_start(selt[:], p["sels"][:])

    cbf = pg.tile([128, 272], BT, tag="cbf", name="cbf")
    cst["ones128"] = cbf[:, 0:128]
    cst["bd16"] = cbf[:, 128:256]
    cst["eye16b"] = cbf[:, 256:272]
    selt = pg.tile([16, 3 * D], BT, tag="sels", name="sels")
    for i, nm in enumerate(("qsel", "cqsel", "bsel")):
        cst[nm] = selt[:, i * D:(i + 1) * D]

    # =====================================================================
    # AdaLN mods: matvec silu(c) @ ada per half-group, strips gathered to
    # [nr, 512] then PE-transposed to feature-major [128, 72].
    # Groups 0-1 run up front; groups 2-8 stream as filler during stage 1.
    # =====================================================================
    mods = pg.tile([128, NMOD * DT], F32, tag="mods", name="mods")
    asmT = pg.tile([8, 3 * 512], BT, tag="asm", name="asmT")
    asmA = asmT[:, 0:512]          # groups 0-1 (rows 0-3)
    asmB = asmT[:, 512:1024]       # groups 2-5 (rows 0-7)
    asmC = asmT[:, 1024:1536]      # groups 6-8 (rows 0-5)

    def ada_group_units(groups, asm, gbase):
        for grp in groups:
            for ch in range(2):
                gi = grp * 2 + ch
                ats = []
                for q in range(2):
                    at = wga(f"ada_g{gi}_{q}")
                    nc.sync.dma_start(at[:], p["ada"][:, gi * 4096 + q * 2048:
                                                      gi * 4096 + (q + 1) * 2048])
                    ats.append(at)
                pm = psum(f"pm{gi}")
                for kp in range(KP):
                    gv = ats[kp // 2][:, (kp % 2) * 1024:(kp % 2 + 1) * 1024]\
                        .rearrange("p (two f) -> p two f", two=2)
                    nc.tensor.matmul(
                        pm[0:1, :], sc8v[:, kp, :, 0:1], gv,
                        start=(kp == 0), stop=(kp == KP - 1),
                        perf_mode=PM.DoubleRow)
                strip = pg.tile([1, 512], BT, tag="strip", name=f"str{gi}",
                                bufs=1)
                nc.scalar.activation(strip[:], pm[0:1, :], ACTF.Identity,
                                     scale=IWS)
                nc.sync.dma_start(asm[gi - 2 * gbase:gi - 2 * gbase + 1, :],
                                  strip[:])
                yield

    def ada_transpose(asm, g0, ng):
        """Transpose an assembly tile's rows into mods columns g0..g0+ng."""
        nr = 2 * ng
        dst = mods[:].rearrange("p (g c k) -> p g c k", c=2, k=4)
        for c4 in range(4):
            pt_ps = ps.tile([128, 1024], BT, tag="ps1", name=f"modsT{g0}_{c4}",
                            bufs=4)
            nc.tensor.transpose(pt_ps[0:128, 0:nr],
                                asm[0:nr, c4 * 128:(c4 + 1) * 128],
                                cst["eye16b"][0:nr, 0:nr])
            src = pt_ps[0:128, 0:nr].rearrange("p (g c) -> p g c", c=2)
            nc.vector.tensor_tensor(
                dst[:, g0:g0 + ng, :, c4], src, cst["adab"].rearrange(
                    "p (g c k) -> p g c k", c=2, k=4)[:, g0:g0 + ng, :, c4],
                ALU.add)

    ada01 = ada_group_units(range(2), asmA, 0)
    next(ada01, None)
    next(ada01, None)
    load_xt_consts()
    for _ in ada01:
        pass
    ada_transpose(asmA, 0, 2)
    ada_mid = ada_group_units(range(2, 6), asmB, 2)
    ada_late = ada_group_units(range(6, NMOD), asmC, 6)

    def msl(i):  # mods columns of modulation param i
        return mods[:, i * DT:(i + 1) * DT]

    def mk_seff(nm, i_scale, w):
        s1 = pg.tile([128, DT], F32, tag=f"seff_{nm}", name=f"seff_{nm}")
        nc.vector.tensor_scalar(s1[:], msl(i_scale), 1.0, None, ALU.add)
        nc.vector.tensor_tensor(s1[:], s1[:], cst[w], ALU.mult)
        return s1

    def mk_gb(nm, i_gate, bias):
        t = pg.tile([128, DT], F32, tag=f"gb_{nm}", name=f"gb_{nm}")
        nc.vector.tensor_tensor(t[:], msl(i_gate), cst[bias], ALU.mult)
        return t

    seff = {"sa": mk_seff("sa", 1, "n1w")}
    gb = {}
    sh_col = {"sa": 0, "ca": 3, "ff": 6}
    g_col = {"sa": 2, "ca": 5, "ff": 8}

    # =====================================================================
    # helpers
    # =====================================================================
    def norm_mod(xtiles, Ttok, seff_t, sh_slice, name, alloc, first=False):
        """RMS + AdaLN modulate of feature-major tiles -> fp8 pair tiles
        (always allocated from the pairw tag)."""
        NCH = Ttok // 512
        pss = [psum(f"ssn_{name}{c}") for c in range(NCH)]
        for k in range(DT):
            for c in range(NCH):
                sq = sqt(f"sq_{name}{k}_{c}")
                sq_engine(k).tensor_tensor(
                    sq[:], xtiles[k][:, c * 512:(c + 1) * 512],
                    xtiles[k][:, c * 512:(c + 1) * 512], ALU.mult)
                nc.tensor.matmul(pss[c][:], cst["ones128"], sq[:],
                                 start=(k == 0), stop=(k == DT - 1))
        rr = scratch4k(f"rr_{name}")
        for c in range(NCH):
            nc.scalar.activation(rr[:, c * 512:(c + 1) * 512], pss[c][:],
                                 ACTF.Sqrt, bias=c_eps[:, 0:1], scale=1.0 / D)
        xn = [pg.tile([128, 2 * Ttok], E4, tag="pairw", name=f"xn_{name}{kp}",
                      bufs=4) for kp in range(KP)]
        # c-chunked: downstream consumers of chunk 0 unblock earlier
        for c in range(NCH):
            cs = slice(c * 512, (c + 1) * 512)
            nc.vector.reciprocal_approx_fast(rr[:, cs], rr[:, cs])
            for k in range(DT):
                tmp = sqt(f"xm_{name}{k}_{c}")
                nc.vector.tensor_tensor(tmp[:], xtiles[k][:, cs],
                                        rr[:, cs], ALU.mult)
                nc.vector.tensor_scalar(
                    xn[k // 2][:, (k % 2) * Ttok + c * 512:
                               (k % 2) * Ttok + (c + 1) * 512],
                    tmp[:], seff_t[:, k:k + 1], sh_slice[:, k:k + 1],
                    ALU.mult, ALU.add)
        return xn

    def qk_norm_start(qtiles, Ttok, selname, name):
        """Per-head RMS norm stats; returns a generator of per-tile apply
        units so callers can interleave them with other PE work."""
        NCH = Ttok // 512
        ssq = scratch4k(f"ssq_{name}", rows=16)
        for c in range(NCH):
            pq = psum(f"psq_{name}{c}")
            for t in range(DT):
                sq = sqt(f"qs_{name}{t}_{c}")
                sq_engine(t).tensor_tensor(
                    sq[:], qtiles[t][:, c * 512:(c + 1) * 512],
                    qtiles[t][:, c * 512:(c + 1) * 512], ALU.mult)
                nc.tensor.matmul(pq[0:16, :],
                                 cst["bd16"][:, t * 16:(t + 1) * 16], sq[:],
                                 start=(t == 0), stop=(t == DT - 1))
            nc.scalar.activation(ssq[:, c * 512:(c + 1) * 512], pq[0:16, :],
                                 ACTF.Sqrt, bias=c_eps[0:16, 0:1], scale=1.0 / HD)
        nc.vector.reciprocal_approx_fast(ssq[:, 0:Ttok], ssq[:, 0:Ttok])
        rqb = scrbf(f"rqb_{name}", wid=Ttok)
        nc.vector.tensor_copy(rqb[:, 0:Ttok], ssq[:, 0:Ttok])

        def apply_units():
            for t in range(DT):
                for c in range(NCH):
                    pb = psum(f"qb_{name}{t}_{c}")
                    nc.tensor.matmul(pb[:],
                                     cst[selname][:, t * 128:(t + 1) * 128],
                                     rqb[:, c * 512:(c + 1) * 512],
                                     start=True, stop=True)
                    nc.vector.tensor_tensor(qtiles[t][:, c * 512:(c + 1) * 512],
                                            qtiles[t][:, c * 512:(c + 1) * 512],
                                            pb[:], ALU.mult)
                yield

        return apply_units()

    def k_norm_units(ktiles, Tk, name, fast=False):
        """Per-head K RMS stats + rk broadcast applied to the k tiles.
        fast=True keeps the whole chain on VectorE (latency-critical,
        pre-attention); fast=False offloads the multiplies to GpSimd via
        an SBUF bounce (used when VectorE is the busier engine)."""
        NCH = Tk // 512
        rss = scratch4k(f"rss_{name}", rows=16)

        def units():
            for c in range(NCH):
                pq = psum(f"psk_{name}{c}")
                for t in range(DT):
                    sq = sqt(f"ks_{name}{t}_{c}")
                    (nc.vector if fast else sq_engine(t)).tensor_tensor(
                        sq[:], ktiles[t][:, c * 512:(c + 1) * 512],
                        ktiles[t][:, c * 512:(c + 1) * 512], ALU.mult)
                    nc.tensor.matmul(pq[0:16, :],
                                     cst["bd16"][:, t * 16:(t + 1) * 16],
                                     sq[:], start=(t == 0), stop=(t == DT - 1))
                nc.scalar.activation(rss[:, c * 512:(c + 1) * 512], pq[0:16, :],
                                     ACTF.Sqrt, bias=c_eps[0:16, 0:1],
                                     scale=1.0 / HD)
                yield
            nc.vector.reciprocal_approx_fast(rss[:, 0:Tk], rss[:, 0:Tk])
            rqb = scrbf(f"rqb_{name}", wid=Tk)
            nc.vector.tensor_copy(rqb[:, 0:Tk], rss[:, 0:Tk])
            yield
            for t in range(DT):
                for c in range(NCH):
                    cs = slice(c * 512, (c + 1) * 512)
                    pb = psum(f"kb_{name}{t}_{c}")
                    nc.tensor.matmul(pb[:],
                                     cst["bsel"][:, t * 128:(t + 1) * 128],
                                     rqb[:, cs], start=True, stop=True)
                    if fast:
                        nc.vector.tensor_tensor(ktiles[t][:, cs],
                                                ktiles[t][:, cs], pb[:],
                                                ALU.mult)
                    else:
                        pbb = sqt(f"pbb_{name}{t}_{c}")
                        nc.vector.tensor_copy(pbb[:], pb[:])
                        nc.gpsimd.tensor_tensor(ktiles[t][:, cs],
                                                ktiles[t][:, cs], pbb[:],
                                                ALU.mult)
                    yield

        return units()

    def attention(q_sb, k_sb, v_sb, Tk, name, filler=None):
        """softmax(q k^T / 8) v.  The score->exp stream is software-
        pipelined ACROSS head-pair boundaries (lookahead 2) so the ACT
        exp chain and the PE PV stream never drain.  Output fp8 k-pair
        tiles holding o/64 (the /64 is folded into the dmask values)."""
        KTk = Tk // 128
        o_pair = [pair8(f"o_{name}{tp}", 2 * T) for tp in range(KP)]

        def o_ap(t, lo=0, hi=128):
            return o_pair[t // 2][lo:hi, (t % 2) * T:(t % 2 + 1) * T]

        dpairs = {}

        def normalize(t):
            pb = psum(f"ob_{name}{t}")
            dpA, dpB = dpairs.pop(t)
            nc.tensor.matmul(pb[:], dmask[0:1, 0:128], dpA[0:1, 0:T],
                             start=True, stop=False)
            nc.tensor.matmul(pb[:], dmask[0:1, 128:256], dpB[0:1, 0:T],
                             start=False, stop=True)
            nc.vector.reciprocal_approx_fast(pb[:], pb[:])
            nc.vector.tensor_tensor(o_ap(t), o_ap(t), pb[:], ALU.mult)

        pts = {}

        def s_unit(t, kt):
            s_ps = psum2(f"s_{name}{t}_{kt}")
            nc.tensor.matmul(
                s_ps[:, 0:512], k_sb[t][0:64, kt * 128:(kt + 1) * 128],
                q_sb[t][0:64, 0:T], start=True, stop=True)
            nc.tensor.matmul(
                s_ps[:, 512:1024], k_sb[t][64:128, kt * 128:(kt + 1) * 128],
                q_sb[t][64:128, 0:T], start=True, stop=True)
            pt = ptile(f"pt_{name}{t}_{kt}")
            nc.scalar.activation(pt[:], s_ps[:], ACTF.Exp, scale=ATT_SCALE)
            pts[(t, kt)] = pt

        units = [(t, kt) for t in range(DT) for kt in range(KTk)]
        s_unit(*units[0])
        s_unit(*units[1])
        poAB = None
        for i, (t, kt) in enumerate(units):
            if kt == 0:
                if t >= 1:
                    normalize(t - 1)
                poAB = [psum(f"po_{name}{2 * t}"),
                        psum(f"po_{name}{2 * t + 1}")]
            if i + 2 < len(units):
                s_unit(*units[i + 2])
            pt = pts.pop((t, kt))
            for half in range(2):
                h16 = 2 * t + half
                nc.tensor.matmul(poAB[half][0:65, :],
                                 v_sb[kt][:, h16 * 65:(h16 + 1) * 65],
                                 pt[:, half * 512:(half + 1) * 512],
                                 start=(kt == 0), stop=(kt == KTk - 1))
            if filler is not None:
                filler(t, kt)
            if kt == KTk - 1:
                dps = []
                for half in range(2):
                    lo = 64 * half
                    # store o/64 in fp8 (unnormalized P@V can exceed fp8
                    # range)
                    nc.vector.tensor_scalar(o_ap(t, lo, lo + 64),
                                            poAB[half][0:64, :],
                                            1.0 / 64.0, None, ALU.mult)
                    dp = pg.tile([1, 512], BT, tag="dstrb",
                                 name=f"dp_{name}{t}_{half}", bufs=2)
                    nc.vector.tensor_copy(dp[0:1, 0:T], poAB[half][64:65, :])
                    dps.append(dp)
                dpairs[t] = dps
        normalize(DT - 1)
        return o_pair

    # ---- projection helpers ----
    def mm_dr(pp, wt, f, xp, c, kp, nkp=KP):
        """One DoubleRow accumulation step."""
        lhsT = wt[kp][:, :, f * 128:(f + 1) * 128]
        rhs = xp[kp][:].rearrange("p (two t) -> p two t", two=2)[
            :, :, c * 512:(c + 1) * 512]
        nc.tensor.matmul(pp[:], lhsT, rhs, start=(kp == 0), stop=(kp == nkp - 1),
                         perf_mode=PM.DoubleRow)

    def proj_fm8(wdram, wcols0, xp, Tt, bias, bias0, name, alloc, n_f=DT,
                 wq=None):
        """fp8-DoubleRow feature-major projection over n_f output tiles."""
        outs = []
        NCH = Tt // 512
        wt = load_w8(wdram, wcols0, n_f * 128 * 2 * KP // 1, f"{name}_w",
                     eng=wq)
        for f in range(n_f):
            o = alloc(f"{name}_o{f}")
            outs.append(o)
            for c in range(NCH):
                pp = psum(f"p_{name}{f}_{c}")
                for kp in range(KP):
                    mm_dr(pp, wt, f, xp, c, kp)
                nc.scalar.activation(
                    o[:, c * 512:(c + 1) * 512], pp[:], ACTF.Identity,
                    bias=bias[:, bias0 + f:bias0 + f + 1],
                    scale=IWS)
        return outs

    def vdst(o, c):
        return o[:, c * 8 * 65:(c * 8 + 8) * 65].rearrange(
            "p (g e) -> p g e", g=8)[:, :, 0:64]

    def vones(o):
        return o[:, 0:16 * 65].rearrange("p (g e) -> p g e", g=16)[:, :, 64:65]

    def proj_tok8(wdram, wcols0, xp, name, interleave=None, wq=None):
        """fp8-DoubleRow token-major V projection (ones cols interleaved)."""
        outs = []
        wt = load_w8(wdram, wcols0, 8192, f"{name}_w", eng=wq)
        for tt in range(N // 128):
            o = vb(f"{name}_v{tt}")
            nc.gpsimd.memset(vones(o), 1.0)
            outs.append(o)
            for c in range(2):
                pp = psum(f"pv_{name}{tt}_{c}")
                for kp in range(KP):
                    lhsT = xp[kp][:].rearrange("p (two t) -> p two t", two=2)[
                        :, :, tt * 128:(tt + 1) * 128]
                    rhs = wt[kp][:, :, c * 512:(c + 1) * 512]
                    nc.tensor.matmul(pp[:], lhsT, rhs, start=(kp == 0),
                                     stop=(kp == KP - 1), perf_mode=PM.DoubleRow)
                nc.scalar.activation(vdst(o, c), pp[:].rearrange(
                    "p (g e) -> p g e", g=8), ACTF.Identity, scale=IWS)
                if interleave is not None:
                    interleave()
        return outs

    def out_proj(wdram, o_in, resid, gcol, gbt, name, outalloc, wq=None):
        """o @ Wo + gated residual add -> f32 tiles."""
        wt = load_w8(wdram, 0, 8192, f"{name}_w", eng=wq)
        outs = []
        for f in range(DT):
            pp = psum(f"p{name}_{f}")
            for kp in range(KP):
                mm_dr(pp, wt, f, o_in, 0, kp)
            xo = outalloc(f"{name}x{f}")
            scol = pg.tile([128, 1], F32, tag="gsc", name=f"gs{name}{f}",
                           bufs=4)
            nc.vector.tensor_scalar(scol[:], gcol[:, f:f + 1], IWS, None,
                                    ALU.mult)
            nc.vector.affine_then_add(xo[:], pp[:], resid[f][:],
                                      scol[:], gbt[:, f:f + 1])
            outs.append(xo)
        return outs

    # =====================================================================
    # Stage 0b: source tokens mega-load; cross-attention K/V projections
    # are emitted as filler inside attention-1.
    # =====================================================================
    srct_mega = pg.tile([128, 2 * KP * M], E4, tag="srctm", name="srctm")
    srct_p = [srct_mega[:, kp * 2 * M:(kp + 1) * 2 * M] for kp in range(KP)]

    def load_srct():
        nc.sync.dma_start(srct_mega[:], p["srct8"][:])

    kca = [kb(f"kca_o{f}") for f in range(DT)]
    vca = []
    for tt in range(M // 128):
        o = vb(f"vca_v{tt}")
        nc.gpsimd.memset(vones(o), 1.0)
        vca.append(o)

    kst_ca = k_norm_units(kca, M, "kca")

    def ckv_units():
        """Generator of one-psum-group units of cross-KV projection work."""
        wt = load_w8(p["wckv"], 0, 8192, "kca_w", eng=(nc.sync, nc.sync))
        for f in range(DT):
            for c in range(2):
                pp = psum(f"p_kca{f}_{c}")
                for kp in range(KP):
                    mm_dr(pp, wt, f, srct_p, c, kp)
                nc.vector.tensor_scalar(kca[f][:, c * 512:(c + 1) * 512],
                                        pp[:], IWS, cst["ckb"][:, f:f + 1],
                                        ALU.mult, ALU.add)
                yield
        yield from kst_ca
        wtv = load_w8(p["wckv"], 8192, 8192, "vca_w",
                      eng=(nc.sync, nc.sync))
        for tt in range(M // 128):
            for c in range(2):
                pp = psum(f"pv_vca{tt}_{c}")
                for kp in range(KP):
                    lhsT = srct_p[kp].rearrange("p (two t) -> p two t", two=2)[
                        :, :, tt * 128:(tt + 1) * 128]
                    rhs = wtv[kp][:, :, c * 512:(c + 1) * 512]
                    nc.tensor.matmul(pp[:], lhsT, rhs, start=(kp == 0),
                                     stop=(kp == KP - 1), perf_mode=PM.DoubleRow)
                nc.vector.tensor_scalar(vdst(vca[tt], c), pp[:].rearrange(
                    "p (g e) -> p g e", g=8), IWS, None, ALU.mult)
                yield

    ckv_box = []

    def a1_filler(t, kt):
        next(ckv_box[0], None)
        if kt % 8 == 7:
            next(ada_mid, None)

    def a2_filler(t, kt):
        if kt % 8 == 7:
            next(ada_late, None)

    # =====================================================================
    # Stage 1: self-attention sublayer
    # =====================================================================
    xn1 = norm_mod(xt_sb, N, seff["sa"], msl(sh_col["sa"]), "n1", kb)
    q_sa = proj_fm8(p["wqkv"], 0, xn1, T, cst["qkvb"], 0, "qsa", qt,
                    wq=(nc.scalar, nc.scalar))
    k_sa = proj_fm8(p["wqkv"], 8192, xn1, N, cst["qkvb"], DT, "ksa", kb,
                    wq=(nc.sync, nc.sync))
    qknq = qk_norm_start(q_sa, T, "qsel", "qsa")
    kst_sa = k_norm_units(k_sa, N, "ksa", fast=True)

    def qk_drain():
        next(qknq, None)
        next(kst_sa, None)
    v_sa = proj_tok8(p["wqkv"], 2 * 8192, xn1, "vsa", interleave=qk_drain,
                     wq=(nc.scalar, nc.scalar))
    for g in (qknq, kst_sa):
        for _ in g:
            pass
    load_srct()
    ckv_box.append(ckv_units())
    xres_sb = []
    for k in range(DT):
        t = xf(f"xres{k}")
        nc.sync.dma_start(t[:], p["xres"][k * 128:(k + 1) * 128, :])
        xres_sb.append(t)
    prewarm(ACTF.Exp)
    o1 = attention(q_sa, k_sa, v_sa, N, "a1", filler=a1_filler)
    for _ in ckv_box[0]:
        pass
    for _ in ada_mid:
        pass
    prewarm(ACTF.Sqrt)
    ada_transpose(asmB, 2, 4)
    gb["sa"] = mk_gb("sa", 2, "obf")
    seff["ca"] = mk_seff("ca", 4, "ncw")
    gb["ca"] = mk_gb("ca", 5, "cobf")

    x1 = out_proj(p["wo"], o1, xres_sb, msl(g_col["sa"]), gb["sa"][:],
                  "o1", xf)

    # =====================================================================
    # Stage 2: cross-attention sublayer
    # =====================================================================
    xnc = norm_mod(x1, T, seff["ca"], msl(sh_col["ca"]), "nc", qt)
    q_ca = proj_fm8(p["wcq"], 0, xnc, T, cst["cqb"], 0, "qca", qt)
    qknqca = qk_norm_start(q_ca, T, "cqsel", "qca")
    # tile-0 q-norm must precede a2 pair 0; later tiles drain in the filler
    next(qknqca, None)

    def a2_filler2(t, kt):
        if kt % 2 == 1:
            next(qknqca, None)
        a2_filler(t, kt)
    prewarm(ACTF.Exp)
    o2 = attention(q_ca, kca, vca, M, "a2", filler=a2_filler2)
    for _ in qknqca:
        pass
    for _ in ada_late:
        pass
    prewarm(ACTF.Sqrt)
    ada_transpose(asmC, 6, 3)
    seff["ff"] = mk_seff("ff", 7, "n2w")
    gb["ff"] = mk_gb("ff", 8, "b3f")
    x2 = out_proj(p["wco"], o2, x1, msl(g_col["ca"]), gb["ca"][:],
                  "o2", xf)

    # =====================================================================
    # Stage 3: SwiGLU FFN sublayer
    # =====================================================================
    xn2 = norm_mod(x2, T, seff["ff"], msl(sh_col["ff"]), "n2", qt)
    prewarm(ACTF.Silu)
    NKH = FHT // 2  # 11 h k-pairs
    h_p8 = [pair8(f"h8_{fp}", 2 * T) for fp in range(NKH)]
    gsc = pg.tile([128, DT], F32, tag="gsc_ff", name="gsc_ff")
    nc.vector.tensor_scalar(gsc[:], msl(g_col["ff"]), IWS, None, ALU.mult)

    w3v = {}

    def load_w3_half(half):
        # 3 chunks: kp 0-3, 4-7, 8-10
        views = []
        for ch, (kp0, nkp) in enumerate(((0, 4), (4, 4), (8, 3))):
            t = w3m(f"w3_{half}_{ch}")
            nc.sync.dma_start(
                t[:, 0:nkp * 1024],
                p["w3"][:, half * 11264 + kp0 * 1024:
                        half * 11264 + (kp0 + nkp) * 1024])
            for k in range(nkp):
                views.append(t[:, k * 1024:(k + 1) * 1024].rearrange(
                    "p (two f) -> p two f", two=2))
        w3v[half] = views

    def w3_mm(half, psf, kp):
        w3t = w3v[half][kp]
        for ff in range(4):
            lhsT = w3t[:, :, ff * 128:(ff + 1) * 128]
            rhs = h_p8[kp][:].rearrange("p (two t) -> p two t", two=2)
            nc.tensor.matmul(psf[ff // 2][:, (ff % 2) * 512:(ff % 2 + 1) * 512],
                             lhsT, rhs, start=(kp == 0),
                             stop=(kp == NKH - 1),
                             perf_mode=PM.DoubleRow)

    def w3_evac(fg, psf):
        for ff in range(4):
            f = fg + ff
            xo = xf(f"xout{f}")
            nc.vector.affine_then_add(
                xo[:], psf[ff // 2][:, (ff % 2) * 512:(ff % 2 + 1) * 512],
                x2[f][:], gsc[:, f:f + 1], gb["ff"][:, f:f + 1])
            nc.sync.dma_start(p["out"][f * 128:(f + 1) * 128, :], xo[:])

    # first w3 output half accumulates as h pairs complete
    load_w3_half(0)
    psfA = [psum2(f"pfA{i}") for i in range(2)]
    for gi, (f0, nf) in enumerate(W1GRP):
        w1t = load_w8(p["w1"], f0 * 128 * 2 * KP, nf * 128 * 2 * KP,
                      f"w1_{f0}", alloc=wff)
        w2t = load_w8(p["w2"], f0 * 128 * 2 * KP, nf * 128 * 2 * KP,
                      f"w2_{f0}", alloc=wff)
        for f in range(nf):
            fa = f0 + f
            pp1 = psum(f"ph1_{fa}")
            for kp in range(KP):
                mm_dr(pp1, w1t, f, xn2, 0, kp)
            h1 = pg.tile([128, T], BT, tag="h_sb", name=f"h_{fa}", bufs=3)
            nc.scalar.activation(h1[:], pp1[:], ACTF.Silu,
                                 bias=cst["b1f"][:, fa:fa + 1], scale=IWS)
            pp2 = psum(f"ph2_{fa}")
            for kp in range(KP):
                mm_dr(pp2, w2t, f, xn2, 0, kp)
            h2 = sqt(f"h2_{fa}")
            nc.vector.tensor_scalar(h2[:], pp2[:], IWS,
                                    cst["b2f"][:, fa:fa + 1],
                                    ALU.mult, ALU.add)
            nc.vector.tensor_tensor(
                h_p8[fa // 2][:, (fa % 2) * T:(fa % 2 + 1) * T],
                h1[:], h2[:], ALU.mult)
            if fa % 2 == 1:
                w3_mm(0, psfA, fa // 2)
            if fa == 19:
                load_w3_half(1)
    w3_evac(0, psfA)
    psfB = [psum2(f"pfB{i}") for i in range(2)]
    for kp in range(NKH):
        w3_mm(1, psfB, kp)
    w3_evac(4, psfB)

    pg.release()
    ps.release()


# ==========================================================================
# host side
# ==========================================================================

def _fm(vec):
    """[128*k] f32 vector -> feature-major [128, k] (col j = feature tile j)."""
    v = np.asarray(vec, np.float32)
    return np.ascontiguousarray(v.reshape(-1, 128).T)


def _bd16():
    bd = np.zeros((128, 128), np.float32)
    for t in range(8):
        for p_ in range(128):
            bd[p_, t * 16 + 2 * t + p_ // 64] = 1.0
    return bd.astype(BF16)


def _sel(weights64):
    """[16, 1024] selector: sel[i, t*128+p] = w[p%64] * (i == 2t + p//64)."""
    w = np.ones(64, np.float32) if weights64 is None else \
        np.asarray(weights64, np.float32)
    s = np.zeros((16, D), np.float32)
    for col in range(D):
        i = 2 * (col // 128) + (col % 128) // 64
        s[i, col] = w[col % 64]
    return s.astype(BF16)


def _pack8(w):
    """[K, F] f32 -> [K//2, 2, F] fp8 k-pair pack, scaled by WS."""
    w = np.asarray(w, np.float32) * WS
    w = np.clip(w, -239.0, 239.0)
    nkp = w.shape[0] // 256
    kp = w.reshape(nkp, 2, 128, -1).transpose(0, 2, 1, 3)  # [nkp,128,2,F]
    return np.ascontiguousarray(kp.reshape(nkp * 128, 2, w.shape[1])).astype(F8)


def _mega8(w, fgroups=None):
    """[K=1024, F] f32 -> [128, ...] device mega, col order (grp,(kp,two,f))."""
    pk = _pack8(w)                      # [KP*128, 2, F]
    F = pk.shape[-1]
    pk = pk.reshape(KP, 128, 2, F).transpose(1, 0, 2, 3)  # [128, KP, 2, F]
    if fgroups is None:
        fgroups = [(0, F // 128)]
    blocks = []
    for f0, nf in fgroups:
        blocks.append(pk[:, :, :, f0 * 128:(f0 + nf) * 128]
                      .reshape(128, KP * 2 * nf * 128))
    return np.ascontiguousarray(np.concatenate(blocks, axis=1))


def make_in_maps(inputs):
    f32 = lambda a: np.ascontiguousarray(np.asarray(a, np.float32))

    x = f32(inputs["x"]); src = f32(inputs["source_tokens"]); c = f32(inputs["c"])
    qkv_b = f32(inputs["sa_qkv_b"])
    o_w = f32(inputs["sa_o_w"]); o_b = f32(inputs["sa_o_b"])
    ckv_b = f32(inputs["ca_kv_b"])
    co_w = f32(inputs["ca_o_w"]); co_b = f32(inputs["ca_o_b"])
    w1 = f32(inputs["mlp_w1"]); b1 = f32(inputs["mlp_b1"])
    w2 = f32(inputs["mlp_w2"]); b2 = f32(inputs["mlp_b2"])
    w3 = f32(inputs["mlp_w3"]); b3 = f32(inputs["mlp_b3"])

    # pad SwiGLU hidden to 2816; zero pads keep silu(0)*0 == 0 exact
    w1p = np.zeros((D, MHP), np.float32); w1p[:, :MH] = w1
    w2p = np.zeros((D, MHP), np.float32); w2p[:, :MH] = w2
    w3p = np.zeros((MHP, D), np.float32); w3p[:MH, :] = w3
    b1p = np.zeros(MHP, np.float32); b1p[:MH] = b1
    b2p = np.zeros(MHP, np.float32); b2p[:MH] = b2

    # fold the V biases through the linear attention + output projection:
    # softmax(..) @ (v + vb) @ Wo = softmax(..) @ v @ Wo + vb @ Wo
    obf = qkv_b[2 * D:3 * D] @ o_w + o_b
    cobf = ckv_b[D:2 * D] @ co_w + co_b

    cf32 = np.zeros((128, CF32_COLS), np.float32)
    for nm, val in (("adab", f32(inputs["ada_b"])), ("n1w", f32(inputs["n1_w"])),
                    ("ncw", f32(inputs["nc_w"])), ("n2w", f32(inputs["n2_w"])),
                    ("qkvb", qkv_b), ("obf", obf),
                    ("cqb", f32(inputs["ca_q_b"])), ("ckb", ckv_b[0:D]),
                    ("cobf", cobf), ("b1f", b1p), ("b2f", b2p), ("b3f", b3)):
        c0, ncol = _CF32[nm]
        cf32[:, c0:c0 + ncol] = _fm(val)

    eye16p = np.zeros((128, 16), np.float32)
    eye16p[:16] = np.eye(16, dtype=np.float32)
    cbf = np.concatenate([np.ones((128, 128), np.float32),
                          _bd16().astype(np.float32), eye16p],
                         axis=1).astype(BF16)
    sels = np.concatenate([
        _sel(np.asarray(inputs["sa_qn_w"], np.float32)
             * np.asarray(inputs["sa_kn_w"], np.float32)).astype(np.float32),
        _sel(np.asarray(inputs["ca_qn_w"], np.float32)
             * np.asarray(inputs["ca_kn_w"], np.float32)).astype(np.float32),
        _sel(None).astype(np.float32),
        ], axis=1).astype(BF16)

    # ada: 18 half-group megas, each [128, KP*2*512]
    ada_f = f32(inputs["ada_w"])
    ada_mega = _mega8(ada_f, fgroups=[(4 * g, 4) for g in range(18)])

    # w3: (half, kp, two, f512) layout
    w3pk = _pack8(w3p)                                  # [11*128, 2, 1024]
    w3pk = w3pk.reshape(11, 128, 2, 1024).transpose(1, 0, 2, 3)
    w3m = np.concatenate(
        [w3pk[:, :, :, h * 512:(h + 1) * 512].reshape(128, 11 * 1024)
         for h in range(2)], axis=1)

    shared = dict(
        ada=ada_mega,
        cf32=cf32, cbf=cbf, sels=sels,
        wqkv=_mega8(f32(inputs["sa_qkv_w"]),
                    fgroups=[(0, 8), (8, 8), (16, 8)]),
        wo=_mega8(o_w), wcq=_mega8(f32(inputs["ca_q_w"])),
        wckv=_mega8(f32(inputs["ca_kv_w"]), fgroups=[(0, 8), (8, 8)]),
        wco=_mega8(co_w),
        w1=_mega8(w1p, fgroups=W1GRP), w2=_mega8(w2p, fgroups=W1GRP),
        w3=np.ascontiguousarray(w3m),
    )

    in_maps = []
    for cidx in range(NCORES):
        b, half = divmod(cidx, 2)
        xT = x[b].T  # [D, N]
        if half:
            xTp = np.concatenate([xT[:, T:], xT[:, :T]], axis=1)
        else:
            xTp = xT
        m = dict(shared)
        xbf = np.ascontiguousarray(xTp).astype(BF16)
        for k in range(DT):
            m[f"xt{k}"] = np.ascontiguousarray(xbf[k * 128:(k + 1) * 128])
        m["xres"] = np.ascontiguousarray(xTp[:, :T])
        s8 = np.clip(np.ascontiguousarray(src[b].T), -239.0, 239.0).astype(F8)
        m["srct8"] = np.ascontiguousarray(
            s8.reshape(KP, 2, 128, M).transpose(2, 0, 1, 3)
            .reshape(128, KP * 2 * M))
        m["cvec"] = np.ascontiguousarray(c[b].reshape(DT, 128).T)
        in_maps.append(m)
    return in_maps


def assemble(results):
    out = np.empty((B, N, D), np.float32)
    for cidx in range(NCORES):
        b, half = divmod(cidx, 2)
        out[b, half * T:(half + 1) * T, :] = results[cidx]["out"].T
    return out


_NC_CACHE = []


def kernel(**inputs):
    from concourse.bass_utils import run_bass_kernel_spmd
    if not _NC_CACHE:
        _NC_CACHE.append(build_graph())
    nc = _NC_CACHE[0]
    in_maps = make_in_maps(inputs)
    res = run_bass_kernel_spmd(nc, in_maps, core_ids=list(range(NCORES)))
    return assemble(res.results)


if __name__ == "__main__":
    nc = build_graph()
    print("graph built OK; instructions:",
          sum(len(bb.instructions) for bb in nc.main_func.blocks))


# revision 29
# speedup vs baseline: 1.0166x; 1.0166x over previous
"""Trainium2 Bass kernel for nn_ConditionalJiTBlock (DiT-style block with
AdaLN modulation, self-attention, cross-attention and SwiGLU FFN).

Sharding: 8 NeuronCores = 4 batch elements x 2 token-halves. Each core
computes its 512 query tokens end-to-end with zero collectives; the K/V
projections (which need all 1024 tokens of the batch element) are
replicated within each pair of cores. SPMD safety: the host permutes each
core's token axis so the core's local tokens are always columns 0..511 of
the on-chip tensors (attention is permutation-invariant over key tokens).

Layout: activations are feature-major on chip (features on partitions,
tokens on the free axis). Projections run as fp8 DoubleRow matmuls;
weights are host-prepacked into device-contiguous [128, cols] megas
(col order kp,two,f) so each load is one or two large contiguous DMAs.
Attention scores for a head pair are emitted as two row-tiled concurrent
K=64 matmuls into one 2-bank [128, 1024] PSUM tile, exponentiated by a
single wide ACT op. Softmax denominators come from an interleaved
ones-column in the token-major V tiles. Startup: a short PE warm-up
burst opens the HAM clock gate while the critical DMAs (cvec, ada
groups 0-1, xt, wqkv) stream in, issued across four engine queues.
"""

import numpy as np
import ml_dtypes

BF16 = ml_dtypes.bfloat16
F8 = ml_dtypes.float8_e4m3

B, N, M, D, H, HD = 4, 1024, 1024, 1024, 16, 64
MH = 2730
MHP = 2816          # MH padded to 22*128
EPS = 1e-6
NCORES = 8
T = 512             # local query tokens per core
DT = D // 128       # 8
KP = DT // 2        # 4 k-tile pairs for DoubleRow
FHT = MHP // 128    # 22
NMOD = 9
ATT_SCALE = HD ** -0.5
WS = 1024.0         # fp8 weight pre-scale (power of 2)
IWS = 1.0 / WS

# cf32 mega-constant column map: name -> (col0, ncols)
_CF32 = {}
_c = 0
for _nm, _nc_ in (("adab", NMOD * DT), ("n1w", DT), ("ncw", DT), ("n2w", DT),
                  ("qkvb", 3 * DT), ("obf", DT), ("cqb", DT), ("ckb", DT),
                  ("cobf", DT), ("b1f", FHT), ("b2f", FHT), ("b3f", DT)):
    _CF32[_nm] = (_c, _nc_)
    _c += _nc_
CF32_COLS = _c

W1GRP = [(0, 4), (4, 4), (8, 4), (12, 4), (16, 4), (20, 2)]   # w1/w2 f-tile groups


# ==========================================================================
# device graph
# ==========================================================================

def build_graph():
    import concourse.bacc as bacc
    import concourse.mybir as mybir
    import concourse.tile as tile

    F32 = mybir.dt.float32
    BT = mybir.dt.bfloat16
    E4 = mybir.dt.float8e4

    nc = bacc.Bacc("TRN2", target_bir_lowering=False, debug=False,
                   num_devices=NCORES)

    def din(name, shape, dtype):
        return nc.dram_tensor(name, shape, dtype, kind="ExternalInput").ap()

    p = {}
    # activations (host-prepacked feature-major / kp-major layouts)
    for k in range(DT):
        p[f"xt{k}"] = din(f"xt{k}", [128, N], BT)       # x[b].T tile k
    p["xres"] = din("xres", [D, T], F32)                # f32 residual columns
    p["cvec"] = din("cvec", [128, DT], F32)             # c[b] feature-major
    p["srct8"] = din("srct8", [128, 2 * KP * M], E4)    # (kp, j, m)
    # weights: fp8 megas, col order (kp, two, f) per group
    p["ada"] = din("ada", [128, 18 * 4096], E4)         # 18 half-groups
    p["wqkv"] = din("wqkv", [128, 3 * 8192], E4)
    p["wo"] = din("wo", [128, 8192], E4)
    p["wcq"] = din("wcq", [128, 8192], E4)
    p["wckv"] = din("wckv", [128, 2 * 8192], E4)
    p["wco"] = din("wco", [128, 8192], E4)
    p["w1"] = din("w1", [128, KP * 2 * MHP], E4)        # groups of 8 f-tiles
    p["w2"] = din("w2", [128, KP * 2 * MHP], E4)
    p["w3"] = din("w3", [128, 2 * 11 * 1024], E4)       # (half, kp, two, f512)
    # constants
    p["cf32"] = din("cf32", [128, CF32_COLS], F32)
    p["cbf"] = din("cbf", [128, 272], BT)               # ones128 | bd16 | eye16
    p["sels"] = din("sels", [16, 3 * D], BT)            # qsel|cqsel|bsel

    p["out"] = nc.dram_tensor("out", [D, T], F32, kind="ExternalOutput").ap()

    with tile.TileContext(nc) as tc:
        _emit(nc, tc, p, mybir)
    nc.compile()
    return nc


def _emit(nc, tc, p, mybir):
    ALU = mybir.AluOpType
    ACTF = mybir.ActivationFunctionType
    PM = mybir.MatmulPerfMode
    F32 = mybir.dt.float32
    BT = mybir.dt.bfloat16
    E4 = mybir.dt.float8e4

    pg = tc.alloc_tile_pool(name="pg", bufs=1)
    ps = tc.alloc_tile_pool(name="ps", bufs=8, space="PSUM")

    # ---- psum allocators: 2x [128,1024] (2 banks) + 4x [128,512] ----
    def psum2(name):
        return ps.tile([128, 1024], F32, tag="ps2", name=name, bufs=2)

    def psum(name):
        return ps.tile([128, 512], F32, tag="ps1", name=name, bufs=4)

    # ---- sbuf allocators ----
    def kb(name):     # bf16 [128,1024] xt/k tiles
        return pg.tile([128, 1024], BT, tag="kb", name=name, bufs=16)

    def vb(name):     # bf16 [128,1040] v tiles (ones cols interleaved)
        return pg.tile([128, 1040], BT, tag="vb", name=name, bufs=16)

    def xf(name):     # f32 [128, T] residual-stream tiles
        return pg.tile([128, T], F32, tag="xf", name=name, bufs=16)

    def qt(name):     # bf16 [128, T] q tiles
        return pg.tile([128, T], BT, tag="qt", name=name, bufs=8)

    def wg4(name):    # fp8 packed weight stream tiles (2 kp each)
        return pg.tile([128, 4096], E4, tag="wg4", name=name, bufs=3)

    def wga(name):    # fp8 ada quarter-group stream tiles (2 kp)
        return pg.tile([128, 2048], E4, tag="wga", name=name, bufs=2)

    def wff(name):    # fp8 w1/w2 stream tiles (half-groups of 2 kp)
        return pg.tile([128, 2048], E4, tag="wff", name=name, bufs=4)

    def w3m(name):    # fp8 w3 chunk tiles
        return pg.tile([128, 4096], E4, tag="w3m", name=name, bufs=3)

    def pairw(name):  # fp8 k-pair tiles, 1024 tokens (xn1)
        return pg.tile([128, 2048], E4, tag="pairw", name=name, bufs=4)

    def pair8(name, wid):  # fp8 k-pair tiles, 512 tokens (xn/o/h)
        return pg.tile([128, 1024], E4, tag="pair8", name=name, bufs=11)

    def ptile(name):  # wide exp(p) tiles
        return pg.tile([128, 1024], BT, tag="pt", name=name, bufs=3)

    def sqt(name):    # square scratch
        return pg.tile([128, 512], BT, tag="sq", name=name, bufs=3)

    def scratch4k(name, rows=128, wid=1024):  # f32 scratch (rr/ssq/den)
        return pg.tile([rows, wid], F32, tag="s4k", name=name, bufs=2)

    def scrbf(name, rows=16, wid=512):
        return pg.tile([rows, wid], BT, tag="sbf", name=name, bufs=2)

    def sq_engine(i):
        return nc.gpsimd if i % 2 == 1 else nc.vector

    # =====================================================================
    # Stage 0: PE warm-up burst + input DMAs in critical-path order,
    # spread across four engine queues; then silu(c).
    # =====================================================================
    warm = pg.tile([128, 512], BT, tag="sq", name="warm", bufs=3)
    nc.vector.memset(warm[:], 0.0)
    dmy = pg.tile([1, 4], F32, tag="dmy", name="dmy")
    wps = psum("warmps")
    for i in range(20):
        nc.tensor.matmul(wps[:], warm[:, 0:128], warm[:], start=True,
                         stop=True)
    nc.vector.tensor_copy(dmy[:, 0:1], wps[0:1, 0:1])

    # critical first: cvec (gates silu(c) -> ada matvec)
    cv = pg.tile([128, DT], F32, tag="cv", name="cv")
    nc.sync.dma_start(cv[:], p["cvec"][:])
    cf32 = pg.tile([128, CF32_COLS], F32, tag="cf32", name="cf32")
    nc.scalar.dma_start(cf32[:], p["cf32"][:])

    cst = {nm: cf32[:, c0:c0 + ncol] for nm, (c0, ncol) in _CF32.items()}

    c_eps = pg.tile([128, 2], F32, tag="c_eps", name="c_eps")
    nc.gpsimd.memset(c_eps[:, 0:1], EPS)
    nc.gpsimd.memset(c_eps[:, 1:2], HD * EPS)
    dmask = pg.tile([1, 256], BT, tag="dmask", name="dmask")
    nc.gpsimd.memset(dmask[:], 0.0)
    nc.gpsimd.memset(dmask[0:1, 0:64], 1.0 / 64.0)
    nc.gpsimd.memset(dmask[0:1, 192:256], 1.0 / 64.0)

    def prewarm(func):
        nc.scalar.activation(dmy[:, 1:2], c_eps[0:1, 0:1], func)

    # weight mega loader: returns per-kp [128, 2, F] views
    def load_w8(dram, col0, ncols, tagname, eng=None, alloc=None):
        F = ncols // (KP * 2)
        alloc = alloc or wg4
        engs = eng or (nc.sync, nc.sync)
        tiles = []
        for h in range(2):
            t = alloc(f"{tagname}_{h}")
            engs[h].dma_start(t[:, 0:ncols // 2],
                              dram[:, col0 + h * ncols // 2:
                                   col0 + (h + 1) * ncols // 2])
            tiles.append(t)
        views = []
        for kp in range(KP):
            base = (kp % 2) * (2 * F)
            views.append(tiles[kp // 2][:, base:base + 2 * F]
                         .rearrange("p (two f) -> p two f", two=2))
        return views

    # silu(c) -> fp8 DoubleRow stationary (emitted before the bulk DMA
    # issues so the scalar queue reaches the sigmoid immediately)
    sc = pg.tile([128, DT], BT, tag="sc", name="sc")
    nc.scalar.activation(sc[:], cv[:], ACTF.Sigmoid)
    prewarm(ACTF.Sqrt)
    nc.vector.tensor_tensor(sc[:], sc[:], cv[:], ALU.mult)
    sc8 = pg.tile([128, 128], E4, tag="sc8", name="sc8")
    sc8v = sc8[:].rearrange("p (kp two s) -> p kp two s", two=2, s=16)
    nc.vector.tensor_copy(sc8v[:, :, :, 0:1],
                          sc[:].rearrange("p (kp two) -> p kp two", two=2)
                          .rearrange("p kp two -> p kp two ()"))

    xt_sb = [kb(f"xt{k}") for k in range(DT)]

    def load_xt_consts():
        for k in range(DT):
            (nc.scalar if k % 2 == 0 else nc.sync).dma_start(
                xt_sb[k][:, 0:N], p[f"xt{k}"][:])
        nc.scalar.dma_start(cbf[:], p["cbf"][:])
        nc.scalar.dma_start(selt[:], p["sels"][:])

    cbf = pg.tile([128, 272], BT, tag="cbf", name="cbf")
    cst["ones128"] = cbf[:, 0:128]
    cst["bd16"] = cbf[:, 128:256]
    cst["eye16b"] = cbf[:, 256:272]
    selt = pg.tile([16, 3 * D], BT, tag="sels", name="sels")
    for i, nm in enumerate(("qsel", "cqsel", "bsel")):
        cst[nm] = selt[:, i * D:(i + 1) * D]

    # =====================================================================
    # AdaLN mods: matvec silu(c) @ ada per half-group, strips gathered to
    # [nr, 512] then PE-transposed to feature-major [128, 72].
    # Groups 0-1 run up front; groups 2-8 stream as filler during stage 1.
    # =====================================================================
    mods = pg.tile([128, NMOD * DT], F32, tag="mods", name="mods")
    asmT = pg.tile([8, 3 * 512], BT, tag="asm", name="asmT")
    asmA = asmT[:, 0:512]          # groups 0-1 (rows 0-3)
    asmB = asmT[:, 512:1024]       # groups 2-5 (rows 0-7)
    asmC = asmT[:, 1024:1536]      # groups 6-8 (rows 0-5)

    def ada_group_units(groups, asm, gbase):
        for grp in groups:
            for ch in range(2):
                gi = grp * 2 + ch
                ats = []
                for q in range(2):
                    at = wga(f"ada_g{gi}_{q}")
                    nc.sync.dma_start(at[:], p["ada"][:, gi * 4096 + q * 2048:
                                                      gi * 4096 + (q + 1) * 2048])
                    ats.append(at)
                pm = psum(f"pm{gi}")
                for kp in range(KP):
                    gv = ats[kp // 2][:, (kp % 2) * 1024:(kp % 2 + 1) * 1024]\
                        .rearrange("p (two f) -> p two f", two=2)
                    nc.tensor.matmul(
                        pm[0:1, :], sc8v[:, kp, :, 0:1], gv,
                        start=(kp == 0), stop=(kp == KP - 1),
                        perf_mode=PM.DoubleRow)
                strip = pg.tile([1, 512], BT, tag="strip", name=f"str{gi}",
                                bufs=1)
                nc.vector.tensor_scalar(strip[:], pm[0:1, :], IWS, None,
                                        ALU.mult)
                nc.sync.dma_start(asm[gi - 2 * gbase:gi - 2 * gbase + 1, :],
                                  strip[:])
                yield

    def ada_transpose(asm, g0, ng):
        """Transpose an assembly tile's rows into mods columns g0..g0+ng."""
        nr = 2 * ng
        dst = mods[:].rearrange("p (g c k) -> p g c k", c=2, k=4)
        for c4 in range(4):
            pt_ps = ps.tile([128, 1024], BT, tag="ps1", name=f"modsT{g0}_{c4}",
                            bufs=4)
            nc.tensor.transpose(pt_ps[0:128, 0:nr],
                                asm[0:nr, c4 * 128:(c4 + 1) * 128],
                                cst["eye16b"][0:nr, 0:nr])
            src = pt_ps[0:128, 0:nr].rearrange("p (g c) -> p g c", c=2)
            nc.vector.tensor_tensor(
                dst[:, g0:g0 + ng, :, c4], src, cst["adab"].rearrange(
                    "p (g c k) -> p g c k", c=2, k=4)[:, g0:g0 + ng, :, c4],
                ALU.add)

    ada01 = ada_group_units(range(2), asmA, 0)
    next(ada01, None)
    next(ada01, None)
    load_xt_consts()
    for _ in ada01:
        pass
    ada_transpose(asmA, 0, 2)
    ada_mid = ada_group_units(range(2, 6), asmB, 2)
    ada_late = ada_group_units(range(6, NMOD), asmC, 6)

    def msl(i):  # mods columns of modulation param i
        return mods[:, i * DT:(i + 1) * DT]

    def mk_seff(nm, i_scale, w):
        s1 = pg.tile([128, DT], F32, tag=f"seff_{nm}", name=f"seff_{nm}")
        nc.vector.tensor_scalar(s1[:], msl(i_scale), 1.0, None, ALU.add)
        nc.vector.tensor_tensor(s1[:], s1[:], cst[w], ALU.mult)
        return s1

    def mk_gb(nm, i_gate, bias):
        t = pg.tile([128, DT], F32, tag=f"gb_{nm}", name=f"gb_{nm}")
        nc.vector.tensor_tensor(t[:], msl(i_gate), cst[bias], ALU.mult)
        return t

    seff = {"sa": mk_seff("sa", 1, "n1w")}
    gb = {}
    sh_col = {"sa": 0, "ca": 3, "ff": 6}
    g_col = {"sa": 2, "ca": 5, "ff": 8}

    # =====================================================================
    # helpers
    # =====================================================================
    def norm_mod(xtiles, Ttok, seff_t, sh_slice, name, alloc, first=False):
        """RMS + AdaLN modulate of feature-major tiles -> fp8 pair tiles
        (always allocated from the pairw tag)."""
        NCH = Ttok // 512
        pss = [psum(f"ssn_{name}{c}") for c in range(NCH)]
        for k in range(DT):
            for c in range(NCH):
                sq = sqt(f"sq_{name}{k}_{c}")
                sq_engine(k).tensor_tensor(
                    sq[:], xtiles[k][:, c * 512:(c + 1) * 512],
                    xtiles[k][:, c * 512:(c + 1) * 512], ALU.mult)
                nc.tensor.matmul(pss[c][:], cst["ones128"], sq[:],
                                 start=(k == 0), stop=(k == DT - 1))
        rr = scratch4k(f"rr_{name}")
        for c in range(NCH):
            nc.scalar.activation(rr[:, c * 512:(c + 1) * 512], pss[c][:],
                                 ACTF.Sqrt, bias=c_eps[:, 0:1], scale=1.0 / D)
        xn = [pg.tile([128, 2 * Ttok], E4, tag="pairw", name=f"xn_{name}{kp}",
                      bufs=4) for kp in range(KP)]
        # c-chunked: downstream consumers of chunk 0 unblock earlier
        for c in range(NCH):
            cs = slice(c * 512, (c + 1) * 512)
            nc.vector.reciprocal_approx_fast(rr[:, cs], rr[:, cs])
            for k in range(DT):
                tmp = sqt(f"xm_{name}{k}_{c}")
                nc.vector.tensor_tensor(tmp[:], xtiles[k][:, cs],
                                        rr[:, cs], ALU.mult)
                nc.vector.tensor_scalar(
                    xn[k // 2][:, (k % 2) * Ttok + c * 512:
                               (k % 2) * Ttok + (c + 1) * 512],
                    tmp[:], seff_t[:, k:k + 1], sh_slice[:, k:k + 1],
                    ALU.mult, ALU.add)
        return xn

    def qk_norm_start(qtiles, Ttok, selname, name):
        """Per-head RMS norm stats; returns a generator of per-tile apply
        units so callers can interleave them with other PE work."""
        NCH = Ttok // 512
        ssq = scratch4k(f"ssq_{name}", rows=16)
        for c in range(NCH):
            pq = psum(f"psq_{name}{c}")
            for t in range(DT):
                sq = sqt(f"qs_{name}{t}_{c}")
                sq_engine(t).tensor_tensor(
                    sq[:], qtiles[t][:, c * 512:(c + 1) * 512],
                    qtiles[t][:, c * 512:(c + 1) * 512], ALU.mult)
                nc.tensor.matmul(pq[0:16, :],
                                 cst["bd16"][:, t * 16:(t + 1) * 16], sq[:],
                                 start=(t == 0), stop=(t == DT - 1))
            nc.scalar.activation(ssq[:, c * 512:(c + 1) * 512], pq[0:16, :],
                                 ACTF.Sqrt, bias=c_eps[0:16, 0:1], scale=1.0 / HD)
        nc.vector.reciprocal_approx_fast(ssq[:, 0:Ttok], ssq[:, 0:Ttok])
        rqb = scrbf(f"rqb_{name}", wid=Ttok)
        nc.vector.tensor_copy(rqb[:, 0:Ttok], ssq[:, 0:Ttok])

        def apply_units():
            for t in range(DT):
                for c in range(NCH):
                    pb = psum(f"qb_{name}{t}_{c}")
                    nc.tensor.matmul(pb[:],
                                     cst[selname][:, t * 128:(t + 1) * 128],
                                     rqb[:, c * 512:(c + 1) * 512],
                                     start=True, stop=True)
                    nc.vector.tensor_tensor(qtiles[t][:, c * 512:(c + 1) * 512],
                                            qtiles[t][:, c * 512:(c + 1) * 512],
                                            pb[:], ALU.mult)
                yield

        return apply_units()

    def k_norm_units(ktiles, Tk, name, fast=False):
        """Per-head K RMS stats + rk broadcast applied to the k tiles.
        fast=True keeps the whole chain on VectorE (latency-critical,
        pre-attention); fast=False offloads the multiplies to GpSimd via
        an SBUF bounce (used when VectorE is the busier engine)."""
        NCH = Tk // 512
        rss = scratch4k(f"rss_{name}", rows=16)

        def units():
            for c in range(NCH):
                pq = psum(f"psk_{name}{c}")
                for t in range(DT):
                    sq = sqt(f"ks_{name}{t}_{c}")
                    (nc.vector if fast else sq_engine(t)).tensor_tensor(
                        sq[:], ktiles[t][:, c * 512:(c + 1) * 512],
                        ktiles[t][:, c * 512:(c + 1) * 512], ALU.mult)
                    nc.tensor.matmul(pq[0:16, :],
                                     cst["bd16"][:, t * 16:(t + 1) * 16],
                                     sq[:], start=(t == 0), stop=(t == DT - 1))
                nc.scalar.activation(rss[:, c * 512:(c + 1) * 512], pq[0:16, :],
                                     ACTF.Sqrt, bias=c_eps[0:16, 0:1],
                                     scale=1.0 / HD)
                yield
            nc.vector.reciprocal_approx_fast(rss[:, 0:Tk], rss[:, 0:Tk])
            rqb = scrbf(f"rqb_{name}", wid=Tk)
            nc.vector.tensor_copy(rqb[:, 0:Tk], rss[:, 0:Tk])
            yield
            for t in range(DT):
                for c in range(NCH):
                    cs = slice(c * 512, (c + 1) * 512)
                    pb = psum(f"kb_{name}{t}_{c}")
                    nc.tensor.matmul(pb[:],
                                     cst["bsel"][:, t * 128:(t + 1) * 128],
                                     rqb[:, cs], start=True, stop=True)
                    if fast:
                        nc.vector.tensor_tensor(ktiles[t][:, cs],
                                                ktiles[t][:, cs], pb[:],
                                                ALU.mult)
                    else:
                        pbb = sqt(f"pbb_{name}{t}_{c}")
                        nc.vector.tensor_copy(pbb[:], pb[:])
                        nc.gpsimd.tensor_tensor(ktiles[t][:, cs],
                                                ktiles[t][:, cs], pbb[:],
                                                ALU.mult)
                    yield

        return units()

    def attention(q_sb, k_sb, v_sb, Tk, name, filler=None):
        """softmax(q k^T / 8) v.  The score->exp stream is software-
        pipelined ACROSS head-pair boundaries (lookahead 2) so the ACT
        exp chain and the PE PV stream never drain.  Output fp8 k-pair
        tiles holding o/64 (the /64 is folded into the dmask values)."""
        KTk = Tk // 128
        o_pair = [pair8(f"o_{name}{tp}", 2 * T) for tp in range(KP)]

        def o_ap(t, lo=0, hi=128):
            return o_pair[t // 2][lo:hi, (t % 2) * T:(t % 2 + 1) * T]

        dpairs = {}

        def normalize(t):
            pb = psum(f"ob_{name}{t}")
            dpA, dpB = dpairs.pop(t)
            nc.tensor.matmul(pb[:], dmask[0:1, 0:128], dpA[0:1, 0:T],
                             start=True, stop=False)
            nc.tensor.matmul(pb[:], dmask[0:1, 128:256], dpB[0:1, 0:T],
                             start=False, stop=True)
            nc.vector.reciprocal_approx_fast(pb[:], pb[:])
            nc.vector.tensor_tensor(o_ap(t), o_ap(t), pb[:], ALU.mult)

        pts = {}

        def s_unit(t, kt):
            s_ps = psum2(f"s_{name}{t}_{kt}")
            nc.tensor.matmul(
                s_ps[:, 0:512], k_sb[t][0:64, kt * 128:(kt + 1) * 128],
                q_sb[t][0:64, 0:T], start=True, stop=True)
            nc.tensor.matmul(
                s_ps[:, 512:1024], k_sb[t][64:128, kt * 128:(kt + 1) * 128],
                q_sb[t][64:128, 0:T], start=True, stop=True)
            pt = ptile(f"pt_{name}{t}_{kt}")
            nc.scalar.activation(pt[:], s_ps[:], ACTF.Exp, scale=ATT_SCALE)
            pts[(t, kt)] = pt

        units = [(t, kt) for t in range(DT) for kt in range(KTk)]
        s_unit(*units[0])
        s_unit(*units[1])
        poAB = None
        for i, (t, kt) in enumerate(units):
            if kt == 0:
                if t >= 1:
                    normalize(t - 1)
                poAB = [psum(f"po_{name}{2 * t}"),
                        psum(f"po_{name}{2 * t + 1}")]
            if i + 2 < len(units):
                s_unit(*units[i + 2])
            pt = pts.pop((t, kt))
            for half in range(2):
                h16 = 2 * t + half
                nc.tensor.matmul(poAB[half][0:65, :],
                                 v_sb[kt][:, h16 * 65:(h16 + 1) * 65],
                                 pt[:, half * 512:(half + 1) * 512],
                                 start=(kt == 0), stop=(kt == KTk - 1))
            if filler is not None:
                filler(t, kt)
            if kt == KTk - 1:
                dps = []
                for half in range(2):
                    lo = 64 * half
                    # store o/64 in fp8 (unnormalized P@V can exceed fp8
                    # range)
                    nc.vector.tensor_scalar(o_ap(t, lo, lo + 64),
                                            poAB[half][0:64, :],
                                            1.0 / 64.0, None, ALU.mult)
                    dp = pg.tile([1, 512], BT, tag="dstrb",
                                 name=f"dp_{name}{t}_{half}", bufs=2)
                    nc.vector.tensor_copy(dp[0:1, 0:T], poAB[half][64:65, :])
                    dps.append(dp)
                dpairs[t] = dps
        normalize(DT - 1)
        return o_pair

    # ---- projection helpers ----
    def mm_dr(pp, wt, f, xp, c, kp, nkp=KP):
        """One DoubleRow accumulation step."""
        lhsT = wt[kp][:, :, f * 128:(f + 1) * 128]
        rhs = xp[kp][:].rearrange("p (two t) -> p two t", two=2)[
            :, :, c * 512:(c + 1) * 512]
        nc.tensor.matmul(pp[:], lhsT, rhs, start=(kp == 0), stop=(kp == nkp - 1),
                         perf_mode=PM.DoubleRow)

    def proj_fm8(wdram, wcols0, xp, Tt, bias, bias0, name, alloc, n_f=DT,
                 wq=None):
        """fp8-DoubleRow feature-major projection over n_f output tiles."""
        outs = []
        NCH = Tt // 512
        wt = load_w8(wdram, wcols0, n_f * 128 * 2 * KP // 1, f"{name}_w",
                     eng=wq)
        for f in range(n_f):
            o = alloc(f"{name}_o{f}")
            outs.append(o)
            for c in range(NCH):
                pp = psum(f"p_{name}{f}_{c}")
                for kp in range(KP):
                    mm_dr(pp, wt, f, xp, c, kp)
                nc.scalar.activation(
                    o[:, c * 512:(c + 1) * 512], pp[:], ACTF.Identity,
                    bias=bias[:, bias0 + f:bias0 + f + 1],
                    scale=IWS)
        return outs

    def vdst(o, c):
        return o[:, c * 8 * 65:(c * 8 + 8) * 65].rearrange(
            "p (g e) -> p g e", g=8)[:, :, 0:64]

    def vones(o):
        return o[:, 0:16 * 65].rearrange("p (g e) -> p g e", g=16)[:, :, 64:65]

    def proj_tok8(wdram, wcols0, xp, name, interleave=None, wq=None):
        """fp8-DoubleRow token-major V projection (ones cols interleaved)."""
        outs = []
        wt = load_w8(wdram, wcols0, 8192, f"{name}_w", eng=wq)
        for tt in range(N // 128):
            o = vb(f"{name}_v{tt}")
            nc.gpsimd.memset(vones(o), 1.0)
            outs.append(o)
            for c in range(2):
                pp = psum(f"pv_{name}{tt}_{c}")
                for kp in range(KP):
                    lhsT = xp[kp][:].rearrange("p (two t) -> p two t", two=2)[
                        :, :, tt * 128:(tt + 1) * 128]
                    rhs = wt[kp][:, :, c * 512:(c + 1) * 512]
                    nc.tensor.matmul(pp[:], lhsT, rhs, start=(kp == 0),
                                     stop=(kp == KP - 1), perf_mode=PM.DoubleRow)
                nc.scalar.activation(vdst(o, c), pp[:].rearrange(
                    "p (g e) -> p g e", g=8), ACTF.Identity, scale=IWS)
                if interleave is not None:
                    interleave()
        return outs

    def out_proj(wdram, o_in, resid, gcol, gbt, name, outalloc, wq=None):
        """o @ Wo + gated residual add -> f32 tiles."""
        wt = load_w8(wdram, 0, 8192, f"{name}_w", eng=wq)
        outs = []
        for f in range(DT):
            pp = psum(f"p{name}_{f}")
            for kp in range(KP):
                mm_dr(pp, wt, f, o_in, 0, kp)
            xo = outalloc(f"{name}x{f}")
            scol = pg.tile([128, 1], F32, tag="gsc", name=f"gs{name}{f}",
                           bufs=4)
            nc.vector.tensor_scalar(scol[:], gcol[:, f:f + 1], IWS, None,
                                    ALU.mult)
            nc.vector.affine_then_add(xo[:], pp[:], resid[f][:],
                                      scol[:], gbt[:, f:f + 1])
            outs.append(xo)
        return outs

    # =====================================================================
    # Stage 0b: source tokens mega-load; cross-attention K/V projections
    # are emitted as filler inside attention-1.
    # =====================================================================
    srct_mega = pg.tile([128, 2 * KP * M], E4, tag="srctm", name="srctm")
    srct_p = [srct_mega[:, kp * 2 * M:(kp + 1) * 2 * M] for kp in range(KP)]

    def load_srct():
        nc.sync.dma_start(srct_mega[:], p["srct8"][:])

    kca = [kb(f"kca_o{f}") for f in range(DT)]
    vca = []
    for tt in range(M // 128):
        o = vb(f"vca_v{tt}")
        nc.gpsimd.memset(vones(o), 1.0)
        vca.append(o)

    kst_ca = k_norm_units(kca, M, "kca")

    def ckv_units():
        """Generator of one-psum-group units of cross-KV projection work."""
        wt = load_w8(p["wckv"], 0, 8192, "kca_w", eng=(nc.sync, nc.sync))
        for f in range(DT):
            for c in range(2):
                pp = psum(f"p_kca{f}_{c}")
                for kp in range(KP):
                    mm_dr(pp, wt, f, srct_p, c, kp)
                nc.vector.tensor_scalar(kca[f][:, c * 512:(c + 1) * 512],
                                        pp[:], IWS, cst["ckb"][:, f:f + 1],
                                        ALU.mult, ALU.add)
                yield
        yield from kst_ca
        wtv = load_w8(p["wckv"], 8192, 8192, "vca_w",
                      eng=(nc.sync, nc.sync))
        for tt in range(M // 128):
            for c in range(2):
                pp = psum(f"pv_vca{tt}_{c}")
                for kp in range(KP):
                    lhsT = srct_p[kp].rearrange("p (two t) -> p two t", two=2)[
                        :, :, tt * 128:(tt + 1) * 128]
                    rhs = wtv[kp][:, :, c * 512:(c + 1) * 512]
                    nc.tensor.matmul(pp[:], lhsT, rhs, start=(kp == 0),
                                     stop=(kp == KP - 1), perf_mode=PM.DoubleRow)
                nc.vector.tensor_scalar(vdst(vca[tt], c), pp[:].rearrange(
                    "p (g e) -> p g e", g=8), IWS, None, ALU.mult)
                yield

    ckv_box = []

    def a1_filler(t, kt):
        next(ckv_box[0], None)
        if kt % 8 == 7:
            next(ada_mid, None)

    def a2_filler(t, kt):
        if kt % 8 == 7:
            next(ada_late, None)

    # =====================================================================
    # Stage 1: self-attention sublayer
    # =====================================================================
    xn1 = norm_mod(xt_sb, N, seff["sa"], msl(sh_col["sa"]), "n1", kb)
    q_sa = proj_fm8(p["wqkv"], 0, xn1, T, cst["qkvb"], 0, "qsa", qt,
                    wq=(nc.scalar, nc.scalar))
    k_sa = proj_fm8(p["wqkv"], 8192, xn1, N, cst["qkvb"], DT, "ksa", kb,
                    wq=(nc.sync, nc.sync))
    qknq = qk_norm_start(q_sa, T, "qsel", "qsa")
    kst_sa = k_norm_units(k_sa, N, "ksa", fast=True)

    def qk_drain():
        next(qknq, None)
        next(kst_sa, None)
    v_sa = proj_tok8(p["wqkv"], 2 * 8192, xn1, "vsa", interleave=qk_drain,
                     wq=(nc.scalar, nc.scalar))
    for g in (qknq, kst_sa):
        for _ in g:
            pass
    load_srct()
    ckv_box.append(ckv_units())
    xres_sb = []
    for k in range(DT):
        t = xf(f"xres{k}")
        nc.sync.dma_start(t[:], p["xres"][k * 128:(k + 1) * 128, :])
        xres_sb.append(t)
    prewarm(ACTF.Exp)
    o1 = attention(q_sa, k_sa, v_sa, N, "a1", filler=a1_filler)
    for _ in ckv_box[0]:
        pass
    for _ in ada_mid:
        pass
    prewarm(ACTF.Sqrt)
    ada_transpose(asmB, 2, 4)
    gb["sa"] = mk_gb("sa", 2, "obf")
    seff["ca"] = mk_seff("ca", 4, "ncw")
    gb["ca"] = mk_gb("ca", 5, "cobf")

    x1 = out_proj(p["wo"], o1, xres_sb, msl(g_col["sa"]), gb["sa"][:],
                  "o1", xf)

    # =====================================================================
    # Stage 2: cross-attention sublayer
    # =====================================================================
    xnc = norm_mod(x1, T, seff["ca"], msl(sh_col["ca"]), "nc", qt)
    q_ca = proj_fm8(p["wcq"], 0, xnc, T, cst["cqb"], 0, "qca", qt)
    qknqca = qk_norm_start(q_ca, T, "cqsel", "qca")
    # tile-0 q-norm must precede a2 pair 0; later tiles drain in the filler
    next(qknqca, None)

    def a2_filler2(t, kt):
        if kt % 2 == 1:
            next(qknqca, None)
        a2_filler(t, kt)
    prewarm(ACTF.Exp)
    o2 = attention(q_ca, kca, vca, M, "a2", filler=a2_filler2)
    for _ in qknqca:
        pass
    for _ in ada_late:
        pass
    prewarm(ACTF.Sqrt)
    ada_transpose(asmC, 6, 3)
    seff["ff"] = mk_seff("ff", 7, "n2w")
    gb["ff"] = mk_gb("ff", 8, "b3f")
    x2 = out_proj(p["wco"], o2, x1, msl(g_col["ca"]), gb["ca"][:],
                  "o2", xf)

    # =====================================================================
    # Stage 3: SwiGLU FFN sublayer
    # =====================================================================
    xn2 = norm_mod(x2, T, seff["ff"], msl(sh_col["ff"]), "n2", qt)
    prewarm(ACTF.Silu)
    NKH = FHT // 2  # 11 h k-pairs
    h_p8 = [pair8(f"h8_{fp}", 2 * T) for fp in range(NKH)]
    gsc = pg.tile([128, DT], F32, tag="gsc_ff", name="gsc_ff")
    nc.vector.tensor_scalar(gsc[:], msl(g_col["ff"]), IWS, None, ALU.mult)

    w3v = {}

    def load_w3_half(half):
        # 3 chunks: kp 0-3, 4-7, 8-10
        views = []
        for ch, (kp0, nkp) in enumerate(((0, 4), (4, 4), (8, 3))):
            t = w3m(f"w3_{half}_{ch}")
            nc.sync.dma_start(
                t[:, 0:nkp * 1024],
                p["w3"][:, half * 11264 + kp0 * 1024:
                        half * 11264 + (kp0 + nkp) * 1024])
            for k in range(nkp):
                views.append(t[:, k * 1024:(k + 1) * 1024].rearrange(
                    "p (two f) -> p two f", two=2))
        w3v[half] = views

    def w3_mm(half, psf, kp):
        w3t = w3v[half][kp]
        for ff in range(4):
            lhsT = w3t[:, :, ff * 128:(ff + 1) * 128]
            rhs = h_p8[kp][:].rearrange("p (two t) -> p two t", two=2)
            nc.tensor.matmul(psf[ff // 2][:, (ff % 2) * 512:(ff % 2 + 1) * 512],
                             lhsT, rhs, start=(kp == 0),
                             stop=(kp == NKH - 1),
                             perf_mode=PM.DoubleRow)

    def w3_evac(fg, psf):
        for ff in range(4):
            f = fg + ff
            xo = xf(f"xout{f}")
            nc.vector.affine_then_add(
                xo[:], psf[ff // 2][:, (ff % 2) * 512:(ff % 2 + 1) * 512],
                x2[f][:], gsc[:, f:f + 1], gb["ff"][:, f:f + 1])
            nc.sync.dma_start(p["out"][f * 128:(f + 1) * 128, :], xo[:])

    # first w3 output half accumulates as h pairs complete
    load_w3_half(0)
    psfA = [psum2(f"pfA{i}") for i in range(2)]
    for gi, (f0, nf) in enumerate(W1GRP):
        w1t = load_w8(p["w1"], f0 * 128 * 2 * KP, nf * 128 * 2 * KP,
                      f"w1_{f0}", alloc=wff)
        w2t = load_w8(p["w2"], f0 * 128 * 2 * KP, nf * 128 * 2 * KP,
                      f"w2_{f0}", alloc=wff)
        for f in range(nf):
            fa = f0 + f
            pp1 = psum(f"ph1_{fa}")
            for kp in range(KP):
                mm_dr(pp1, w1t, f, xn2, 0, kp)
            h1 = pg.tile([128, T], BT, tag="h_sb", name=f"h_{fa}", bufs=3)
            nc.scalar.activation(h1[:], pp1[:], ACTF.Silu,
                                 bias=cst["b1f"][:, fa:fa + 1], scale=IWS)
            pp2 = psum(f"ph2_{fa}")
            for kp in range(KP):
                mm_dr(pp2, w2t, f, xn2, 0, kp)
            h2 = sqt(f"h2_{fa}")
            nc.vector.tensor_scalar(h2[:], pp2[:], IWS,
                                    cst["b2f"][:, fa:fa + 1],
                                    ALU.mult, ALU.add)
            nc.vector.tensor_tensor(
                h_p8[fa // 2][:, (fa % 2) * T:(fa % 2 + 1) * T],
                h1[:], h2[:], ALU.mult)
            if fa % 2 == 1:
                w3_mm(0, psfA, fa // 2)
            if fa == 19:
                load_w3_half(1)
    w3_evac(0, psfA)
    psfB = [psum2(f"pfB{i}") for i in range(2)]
    for kp in range(NKH):
        w3_mm(1, psfB, kp)
    w3_evac(4, psfB)

    pg.release()
    ps.release()


# ==========================================================================
# host side
# ==========================================================================

def _fm(vec):
    """[128*k] f32 vector -> feature-major [128, k] (col j = feature tile j)."""
    v = np.asarray(vec, np.float32)
    return np.ascontiguousarray(v.reshape(-1, 128).T)


def _bd16():
    bd = np.zeros((128, 128), np.float32)
    for t in range(8):
        for p_ in range(128):
            bd[p_, t * 16 + 2 * t + p_ // 64] = 1.0
    return bd.astype(BF16)


def _sel(weights64):
    """[16, 1024] selector: sel[i, t*128+p] = w[p%64] * (i == 2t + p//64)."""
    w = np.ones(64, np.float32) if weights64 is None else \
        np.asarray(weights64, np.float32)
    s = np.zeros((16, D), np.float32)
    for col in range(D):
        i = 2 * (col // 128) + (col % 128) // 64
        s[i, col] = w[col % 64]
    return s.astype(BF16)


def _pack8(w):
    """[K, F] f32 -> [K//2, 2, F] fp8 k-pair pack, scaled by WS."""
    w = np.asarray(w, np.float32) * WS
    w = np.clip(w, -239.0, 239.0)
    nkp = w.shape[0] // 256
    kp = w.reshape(nkp, 2, 128, -1).transpose(0, 2, 1, 3)  # [nkp,128,2,F]
    return np.ascontiguousarray(kp.reshape(nkp * 128, 2, w.shape[1])).astype(F8)


def _mega8(w, fgroups=None):
    """[K=1024, F] f32 -> [128, ...] device mega, col order (grp,(kp,two,f))."""
    pk = _pack8(w)                      # [KP*128, 2, F]
    F = pk.shape[-1]
    pk = pk.reshape(KP, 128, 2, F).transpose(1, 0, 2, 3)  # [128, KP, 2, F]
    if fgroups is None:
        fgroups = [(0, F // 128)]
    blocks = []
    for f0, nf in fgroups:
        blocks.append(pk[:, :, :, f0 * 128:(f0 + nf) * 128]
                      .reshape(128, KP * 2 * nf * 128))
    return np.ascontiguousarray(np.concatenate(blocks, axis=1))


def make_in_maps(inputs):
    f32 = lambda a: np.ascontiguousarray(np.asarray(a, np.float32))

    x = f32(inputs["x"]); src = f32(inputs["source_tokens"]); c = f32(inputs["c"])
    qkv_b = f32(inputs["sa_qkv_b"])
    o_w = f32(inputs["sa_o_w"]); o_b = f32(inputs["sa_o_b"])
    ckv_b = f32(inputs["ca_kv_b"])
    co_w = f32(inputs["ca_o_w"]); co_b = f32(inputs["ca_o_b"])
    w1 = f32(inputs["mlp_w1"]); b1 = f32(inputs["mlp_b1"])
    w2 = f32(inputs["mlp_w2"]); b2 = f32(inputs["mlp_b2"])
    w3 = f32(inputs["mlp_w3"]); b3 = f32(inputs["mlp_b3"])

    # pad SwiGLU hidden to 2816; zero pads keep silu(0)*0 == 0 exact
    w1p = np.zeros((D, MHP), np.float32); w1p[:, :MH] = w1
    w2p = np.zeros((D, MHP), np.float32); w2p[:, :MH] = w2
    w3p = np.zeros((MHP, D), np.float32); w3p[:MH, :] = w3
    b1p = np.zeros(MHP, np.float32); b1p[:MH] = b1
    b2p = np.zeros(MHP, np.float32); b2p[:MH] = b2

    # fold the V biases through the linear attention + output projection:
    # softmax(..) @ (v + vb) @ Wo = softmax(..) @ v @ Wo + vb @ Wo
    obf = qkv_b[2 * D:3 * D] @ o_w + o_b
    cobf = ckv_b[D:2 * D] @ co_w + co_b

    cf32 = np.zeros((128, CF32_COLS), np.float32)
    for nm, val in (("adab", f32(inputs["ada_b"])), ("n1w", f32(inputs["n1_w"])),
                    ("ncw", f32(inputs["nc_w"])), ("n2w", f32(inputs["n2_w"])),
                    ("qkvb", qkv_b), ("obf", obf),
                    ("cqb", f32(inputs["ca_q_b"])), ("ckb", ckv_b[0:D]),
                    ("cobf", cobf), ("b1f", b1p), ("b2f", b2p), ("b3f", b3)):
        c0, ncol = _CF32[nm]
        cf32[:, c0:c0 + ncol] = _fm(val)

    eye16p = np.zeros((128, 16), np.float32)
    eye16p[:16] = np.eye(16, dtype=np.float32)
    cbf = np.concatenate([np.ones((128, 128), np.float32),
                          _bd16().astype(np.float32), eye16p],
                         axis=1).astype(BF16)
    sels = np.concatenate([
        _sel(np.asarray(inputs["sa_qn_w"], np.float32)
             * np.asarray(inputs["sa_kn_w"], np.float32)).astype(np.float32),
        _sel(np.asarray(inputs["ca_qn_w"], np.float32)
             * np.asarray(inputs["ca_kn_w"], np.float32)).astype(np.float32),
        _sel(None).astype(np.float32),
        ], axis=1).astype(BF16)

    # ada: 18 half-group megas, each [128, KP*2*512]
    ada_f = f32(inputs["ada_w"])
    ada_mega = _mega8(ada_f, fgroups=[(4 * g, 4) for g in range(18)])

    # w3: (half, kp, two, f512) layout
    w3pk = _pack8(w3p)                                  # [11*128, 2, 1024]
    w3pk = w3pk.reshape(11, 128, 2, 1024).transpose(1, 0, 2, 3)
    w3m = np.concatenate(
        [w3pk[:, :, :, h * 512:(h + 1) * 512].reshape(128, 11 * 1024)
         for h in range(2)], axis=1)

    shared = dict(
        ada=ada_mega,
        cf32=cf32, cbf=cbf, sels=sels,
        wqkv=_mega8(f32(inputs["sa_qkv_w"]),
                    fgroups=[(0, 8), (8, 8), (16, 8)]),
        wo=_mega8(o_w), wcq=_mega8(f32(inputs["ca_q_w"])),
        wckv=_mega8(f32(inputs["ca_kv_w"]), fgroups=[(0, 8), (8, 8)]),
        wco=_mega8(co_w),
        w1=_mega8(w1p, fgroups=W1GRP), w2=_mega8(w2p, fgroups=W1GRP),
        w3=np.ascontiguousarray(w3m),
    )

    in_maps = []
    for cidx in range(NCORES):
        b, half = divmod(cidx, 2)
        xT = x[b].T  # [D, N]
        if half:
            xTp = np.concatenate([xT[:, T:], xT[:, :T]], axis=1)
        else:
            xTp = xT
        m = dict(shared)
        xbf = np.ascontiguousarray(xTp).astype(BF16)
        for k in range(DT):
            m[f"xt{k}"] = np.ascontiguousarray(xbf[k * 128:(k + 1) * 128])
        m["xres"] = np.ascontiguousarray(xTp[:, :T])
        s8 = np.clip(np.ascontiguousarray(src[b].T), -239.0, 239.0).astype(F8)
        m["srct8"] = np.ascontiguousarray(
            s8.reshape(KP, 2, 128, M).transpose(2, 0, 1, 3)
            .reshape(128, KP * 2 * M))
        m["cvec"] = np.ascontiguousarray(c[b].reshape(DT, 128).T)
        in_maps.append(m)
    return in_maps


def assemble(results):
    out = np.empty((B, N, D), np.float32)
    for cidx in range(NCORES):
        b, half = divmod(cidx, 2)
        out[b, half * T:(half + 1) * T, :] = results[cidx]["out"].T
    return out


_NC_CACHE = []


def kernel(**inputs):
    from concourse.bass_utils import run_bass_kernel_spmd
    if not _NC_CACHE:
        _NC_CACHE.append(build_graph())
    nc = _NC_CACHE[0]
    in_maps = make_in_maps(inputs)
    res = run_bass_kernel_spmd(nc, in_maps, core_ids=list(range(NCORES)))
    return assemble(res.results)


if __name__ == "__main__":
    nc = build_graph()
    print("graph built OK; instructions:",
          sum(len(bb.instructions) for bb in nc.main_func.blocks))
